# revision 1
# baseline (speedup 1.0000x reference)
"""Trainium2 Bass kernel for nn_Attention4DDownsample.

Sharding: data-parallel over batch B=64 across 8 cores (8 batches/core).
All parameters replicated. Device program per batch (software-pipelined
across batches: head(b+1) is emitted inside tail(b) so the in-order PE
queue always has independent work during the DVE/DMA tail chains):
  head: x load + polyphase padded planes (built on device),
        q = dwconv+pool+1x1 fused as 27 PSUM-accumulating matmuls,
        k = fold_bn(k_w) @ x, vT = x^T @ v_w^T (+ ones col, per-chunk)
  per head-group hg (4 heads) and k-chunk c (7 x 112):
    S^T[c] = k^T q (K=16) ++ bias via rank-16 U@Mr^T matmul (PSUM accum)
    P[c]   = exp(S^T[c])   (ACT, bf16 out)
    o     += vT[c]^T @ P[c] (PSUM accum; row 64 = softmax denominator)
    + filler PE work riding the ACT-paced loop: hg0 carries the vc
      channel-major transposes of vT (identity matmuls), hg1 the v_local
      dwconv tiles (9 diag-matmuls each)
  normalize: 1/den on partition 64, broadcast via a matmul whose lhsT and
  rhs both sit at partition base 64 (no partition-shift DMA), o *= bcast
  R = relu(o_n + v_local); out = fold(p_w) @ R + b  (bf16)

Host side keeps a persistent AOT-compiled (fast-dispatch) PJRT executable,
device-resident replicated constants (re-uploaded only when the weight
inputs actually change), and recycles the previous call's device output as
the next call's donated output buffer, so a steady-state call ships only
the activation tensor x and reads back the bf16 output.
"""

import os
import sys
import time

for p in ("/opt/trn_rl_repo",):
    if p not in sys.path and os.path.isdir(p):
        sys.path.insert(0, p)
os.environ.setdefault("MYCRO_LOCAL_CACHE", "1")

import numpy as np
import ml_dtypes

import jax
from jax.sharding import Mesh, PartitionSpec, NamedSharding
from jax.experimental.shard_map import shard_map

import concourse.bass as bass
import concourse.mybir as mybir
import concourse.tile as tile
from concourse import bacc
from concourse.bass2jax import (
    _bass_exec_p,
    install_neuronx_cc_hook,
    partition_id_tensor,
    fast_dispatch_compile,
)

BF16 = mybir.dt.bfloat16
F32 = mybir.dt.float32
AF = mybir.ActivationFunctionType
ALU = mybir.AluOpType

N_CORES = 8
B = 64             # full batch
B_LOC = 8          # batches per core
C = 384            # input channels
H = W = 28
N = H * W          # 784 key positions
H2 = W2 = 14
N2 = H2 * W2       # 196 query positions
NH = 8             # heads
KD = 16            # head dim (qk)
DH = 512           # v channels
VD = 64            # v head dim
OUT = 384          # output channels
NCH = 7            # k-position chunks
CHK = 112          # chunk size (7*112 = 784)

bf = ml_dtypes.bfloat16


# ----------------------------------------------------------------------------
# host-side constant prep (bicubic matrices are shape-deterministic)
# ----------------------------------------------------------------------------
_A_CUBIC = -0.75


def _cubic_kernel(x):
    A = _A_CUBIC
    x = np.abs(x)
    return np.where(
        x <= 1.0,
        ((A + 2.0) * x - (A + 3.0)) * x * x + 1.0,
        np.where(x < 2.0, ((A * x - 5.0 * A) * x + 8.0 * A) * x - 4.0 * A, 0.0),
    ).astype(np.float32)


def _bicubic_matrix(out_size, in_size):
    i = np.arange(out_size)
    s = (i + 0.5) * in_size / out_size - 0.5
    i0 = np.floor(s).astype(np.int64)
    t = s - i0
    M = np.zeros((out_size, in_size), np.float32)
    for o in (-1, 0, 1, 2):
        idx = np.clip(i0 + o, 0, in_size - 1)
        np.add.at(M, (i, idx), _cubic_kernel(t - o))
    return M


_WEIGHT_NAMES = (
    "q_local_w", "q_local_b", "q_proj_w", "q_proj_b", "q_bn_s", "q_bn_b",
    "k_w", "k_b", "k_bn_s", "k_bn_b", "v_w", "v_b", "v_bn_s", "v_bn_b",
    "vl_w", "vl_b", "vl_bn_s", "vl_bn_b", "p_w", "p_b", "p_bn_s", "p_bn_b",
    "ab_table", "bias_idxs",
)


def _prep_consts(inputs):
    """Fold BNs/scales into weights, build transposed/bias/diag tensors."""
    f = {k: np.asarray(inputs[k], np.float32) for k in _WEIGHT_NAMES
         if k != "bias_idxs"}
    bias_idxs = np.asarray(inputs["bias_idxs"])

    scale = KD ** -0.5
    # q: q = scale * bn(q_proj @ (dwconv_aug(x) + q_local_b))
    qw = (f["q_bn_s"][:, None] * f["q_proj_w"]) * scale       # [128, 384]
    qb = scale * (f["q_bn_s"] * f["q_proj_b"] + f["q_bn_b"])  # [128]
    qb = qb + qw @ f["q_local_b"]                              # fold dw bias
    kw = f["k_bn_s"][:, None] * f["k_w"]
    kb = f["k_bn_s"] * f["k_b"] + f["k_bn_b"]
    vw = f["v_bn_s"][:, None] * f["v_w"]                       # [512, 384]
    vbeta = f["v_bn_s"] * f["v_b"] + f["v_bn_b"]               # [512]
    # v_local = bn_vl(dwconv(v0 + vbeta, vl_w) + vl_b); o gets +vbeta after
    # normalization. Fold everything constant into one per-channel bias.
    vlw = f["vl_bn_s"][:, None, None] * f["vl_w"][:, 0]        # [512, 3, 3]
    tapsum = f["vl_w"][:, 0].sum(axis=(1, 2))                  # [512]
    vlb = (f["vl_bn_s"] * (vbeta * tapsum + f["vl_b"]) + f["vl_bn_b"]
           + vbeta)                                            # [512]
    pw = f["p_bn_s"][:, None] * f["p_w"]                       # [384, 512]
    pb = f["p_bn_s"] * f["p_b"] + f["p_bn_b"]                  # [384]

    # q dwconv weights with the avgpool folded in as +1 on the center tap
    qlw = f["q_local_w"][:, 0].copy()                          # [384, 3, 3]
    qlw[:, 1, 1] += 1.0

    # attention bias, rank-16 factorization: bias^T_h = U_h @ Mr^T,
    # U_h = Mc @ ab_h^T  [784, 16]
    ab = f["ab_table"][:, bias_idxs]                           # [8, 16, 49]
    Mr = _bicubic_matrix(N2, 16)                               # [196, 16]
    Mc = _bicubic_matrix(N, 49)                                # [784, 49]
    # x is shipped phase-reordered (4 stride-2 planes concatenated); the
    # attention is permutation-invariant over key positions as long as the
    # bias factor U is permuted identically.
    perm = []
    for pr in range(2):
        for pc in range(2):
            for r in range(14):
                for cc2 in range(14):
                    perm.append((2 * r + pr) * W + (2 * cc2 + pc))
    perm = np.asarray(perm)
    UT = np.zeros((128, N), np.float32)                        # rows 16h+j
    for h in range(NH):
        U = (Mc @ ab[h].T)[perm]                               # [784, 16]
        UT[16 * h:16 * h + 16] = U.T

    # Combined S^T lhsT layout: kcomb_hg = [k rows | U rows] where for
    # hg=0: rows 0-63 = k heads 0-3, rows 64-127 = U heads 0-3; for hg=1
    # mirrored (U heads 4-7 in rows 0-63, k heads 4-7 in rows 64-127) so
    # the dynamic k half lands on its natural partition range. The rhs
    # qmu_hg[h] masks both q (head rows) and Mr^T (bias rank rows).
    qmu_init = np.zeros((2, 128, 4, N2), np.float32)
    for hh in range(4):
        qmu_init[0, 64 + 16 * hh:80 + 16 * hh, hh] = Mr.T   # bias rows hg0
        qmu_init[1, 16 * hh:16 * hh + 16, hh] = Mr.T        # bias rows hg1

    # q dwconv folded into the q projection: for input-channel block j and
    # tap (a,b), lhsT[c, o] = qwT[c, o] * qlw[c, a, b] -- the 27 matmuls
    # accumulate q = sum_tap qw @ (w_tap * x_shift_tap) directly in PSUM.
    qdw = np.zeros((3, 9, 128, 128), np.float32)
    qwT_full = qw.T                                            # [384, 128]
    for j in range(3):
        for a in range(3):
            for b in range(3):
                blk = slice(128 * j, 128 * j + 128)
                qdw[j, 3 * a + b] = (qwT_full[blk]
                                     * qlw[blk, a, b][:, None])
    # diag matrices for the v_local depthwise conv
    vd = np.zeros((4, 9, 128, 128), np.float32)
    for t in range(4):
        for a in range(3):
            for b in range(3):
                np.fill_diagonal(vd[t, 3 * a + b],
                                 vlw[128 * t:128 * t + 128, a, b])

    # per-partition bias pack [128, 9]:
    # col 0: kb, 1: qb, 2-5: vlb (4 ptiles), 6-8: pb (3 ptiles)
    bias_pack = np.zeros((128, 9), np.float32)
    bias_pack[:, 0] = kb
    bias_pack[:, 1] = qb
    for t in range(4):
        bias_pack[:, 2 + t] = vlb[128 * t:128 * t + 128]
    for m in range(3):
        bias_pack[:, 6 + m] = pb[128 * m:128 * m + 128]

    consts = {
        "qmu_init": qmu_init.astype(bf),                       # [2,128,4,196]
        "kwT": np.ascontiguousarray(kw.T).astype(bf),          # [384, 128]
        "vwT": np.ascontiguousarray(vw.T).astype(bf),          # [384, 512]
        "pwT": np.ascontiguousarray(pw.T).astype(bf),          # [512, 384]
        "ut": UT.astype(bf),                                   # [128, 784]
        "qd": qdw.astype(bf),                                  # [3,9,128,128]
        "vd": vd.astype(bf),                                   # [4,9,128,128]
        "ident": np.eye(CHK, dtype=np.float32).astype(bf),     # [112, 112]
        "bias_pack": bias_pack,                                # [128, 9] f32
    }
    return consts


# ----------------------------------------------------------------------------
# device program
# ----------------------------------------------------------------------------
def build_program():
    nc = bacc.Bacc()
    x_d = nc.declare_dram_parameter("x", [B_LOC, C, N], BF16, isOutput=False)
    kwT_d = nc.declare_dram_parameter("kwT", [C, 128], BF16, isOutput=False)
    vwT_d = nc.declare_dram_parameter("vwT", [C, DH], BF16, isOutput=False)
    pwT_d = nc.declare_dram_parameter("pwT", [DH, OUT], BF16, isOutput=False)
    ut_d = nc.declare_dram_parameter("ut", [128, N], BF16, isOutput=False)
    qmu_d = nc.declare_dram_parameter("qmu_init", [2, 128, 4, N2], BF16,
                                      isOutput=False)
    qd_d = nc.declare_dram_parameter("qd", [3, 9, 128, 128], BF16,
                                     isOutput=False)
    vd_d = nc.declare_dram_parameter("vd", [4, 9, 128, 128], BF16,
                                     isOutput=False)
    id_d = nc.declare_dram_parameter("ident", [CHK, CHK], BF16,
                                     isOutput=False)
    bias_d = nc.declare_dram_parameter("bias_pack", [128, 9], F32,
                                       isOutput=False)
    out_d = nc.declare_dram_parameter("out", [B_LOC, OUT, N2], BF16,
                                      isOutput=True)

    with tile.TileContext(nc) as tc:
        _emit(nc, tc, x_d, kwT_d, vwT_d, pwT_d, ut_d,
              qmu_d, qd_d, vd_d, id_d, bias_d, out_d)
    nc.finalize()
    return nc


def _emit(nc, tc, x_d, kwT_d, vwT_d, pwT_d, ut_d,
          qmu_d, qd_d, vd_d, id_d, bias_d, out_d):
    from contextlib import ExitStack
    ctx = ExitStack()
    cp = ctx.enter_context(tc.tile_pool(name="consts", bufs=1))
    xp = ctx.enter_context(tc.tile_pool(name="xp", bufs=4))
    sb = ctx.enter_context(tc.tile_pool(name="sb", bufs=3))
    ep = ctx.enter_context(tc.tile_pool(name="ep", bufs=6))
    pp = ctx.enter_context(tc.tile_pool(name="pp", bufs=2, space="PSUM"))
    sp = ctx.enter_context(tc.tile_pool(name="sp", bufs=2, space="PSUM"))
    op = ctx.enter_context(tc.tile_pool(name="op", bufs=2, space="PSUM"))

    # ---- load constants (head(0)-critical tensors first) -------------------
    qd = cp.tile([128, 27, 128], BF16)
    nc.sync.dma_start(qd[:], qd_d.rearrange("t k p m -> p (t k) m"))
    vwT = cp.tile([128, 3, DH], BF16)
    nc.sync.dma_start(vwT[:], vwT_d.rearrange("(j p) m -> p j m", p=128))
    kwT = cp.tile([128, 3, 128], BF16)
    nc.sync.dma_start(kwT[:], kwT_d.rearrange("(j p) m -> p j m", p=128))
    bias = cp.tile([128, 9], F32)
    nc.sync.dma_start(bias[:], bias_d[:])
    # persistent (batch-parity double buffered) combined lhsT and masked rhs
    kcomb = {}
    qmu = {}
    for par in range(2):
        for hg in range(2):
            kt = cp.tile([128, N], BF16, tag=f"kc{par}{hg}")
            # static U half: hg0 -> rows 64-127, hg1 -> rows 0-63
            if hg == 0:
                nc.sync.dma_start(kt[64:128, :], ut_d[0:64, :])
            else:
                nc.sync.dma_start(kt[0:64, :], ut_d[64:128, :])
            kcomb[(par, hg)] = kt
            qt = cp.tile([128, 4, N2], BF16, tag=f"qm{par}{hg}")
            nc.sync.dma_start(qt[:], qmu_d[hg])
            qmu[(par, hg)] = qt
    ident = cp.tile([CHK, CHK], BF16)
    nc.sync.dma_start(ident[:], id_d[:])
    vd = cp.tile([128, 36, 128], BF16)
    nc.sync.dma_start(vd[:], vd_d.rearrange("t k p m -> p (t k) m"))
    pwT = cp.tile([128, 4, OUT], BF16)
    nc.sync.dma_start(pwT[:], pwT_d.rearrange("(j p) m -> p j m", p=128))
    ones = cp.tile([128, VD], BF16)
    nc.vector.memset(ones[:], 1.0)
    vt_ab = []
    for i in range(2):
        t = cp.tile([CHK, NCH, NH, VD + 1], BF16, tag=f"vt{i}")
        for c in range(NCH):
            nc.vector.memset(t[:, c, :, VD:VD + 1], 1.0)
        vt_ab.append(t)
    # polyphase padded planes of x, built on device (pads zeroed once):
    # plane (pr,pc) of chunk j: 15x15, pad row0/col0 zero, data
    # P[r+1,c+1] = x[2r+pr, 2c+pc]. A 3x3/stride-2 tap is then ONE
    # contiguous 209-elem run at offset 15*(dr+1)+(dc+1) (dr,dc in
    # {-1,0}) -- the pad col absorbs the row wrap with zeros.
    xph_ab = []
    for i in range(2):
        t = cp.tile([128, 12, 225], BF16, tag=f"xph{i}")
        nc.gpsimd.memset(t[:], 0.0)
        xph_ab.append(t)
    # vc planes likewise live in persistent parity buffers with zero pads
    vc_ab = []
    for i in range(2):
        t = cp.tile([128, 4, 900], BF16, tag=f"vc{i}")
        nc.gpsimd.memset(t[:], 0.0)
        vc_ab.append(t)

    def tap_geom(a):
        # returns the phase index component for row or col tap index
        return 1 if a == 0 else 0 if a == 1 else 1

    def head(b):
        """x load + polyphase scatter + q dwconv/proj + k + vT for batch b."""
        x = xp.tile([128, 3, N], BF16, tag="x")
        nc.sync.dma_start(x[:], x_d[b].rearrange("(j p) m -> p j m", p=128))
        xph = xph_ab[b % 2]
        # scatter the 12 14x14 planes into the zero-padded 15x15 layout
        nc.gpsimd.tensor_copy(
            xph.rearrange("p g (r c) -> p g r c", r=15)[:, :, 1:15, 1:15],
            x.rearrange("p j (s r c) -> p (j s) r c", s=4, r=14))

        kc0 = kcomb[(b % 2, 0)]
        kc1 = kcomb[(b % 2, 1)]
        qm0 = qmu[(b % 2, 0)]
        qm1 = qmu[(b % 2, 1)]
        # q dwconv with the 1x1 projection folded in: 27 matmuls accumulate
        # q = sum_{j,tap} (qw_j * w_tap) @ xph_shift straight into PSUM
        q_sb = sb.tile([128, N2], BF16, tag="q")
        ps = pp.tile([128, 256], F32, tag="proj")
        for j in range(3):
            for ti in range(9):
                a, bb = divmod(ti, 3)
                pr, pc = tap_geom(a), tap_geom(bb)
                off = (15 if a else 0) + (1 if bb else 0)
                nc.tensor.matmul(
                    ps[:, 0:209], qd[:, 9 * j + ti, :],
                    xph[:, 4 * j + 2 * pr + pc, off:off + 209],
                    start=(j == 0 and ti == 0), stop=(j == 2 and ti == 8))
        nc.vector.tensor_scalar_add(
            q_sb.rearrange("p (r c) -> p r c", r=14),
            ps[:, 0:210].rearrange("p (r c) -> p r c", c=15)[:, :, 0:14],
            bias[:, 1:2])
        for hh in range(4):
            nc.sync.dma_start(qm0[16 * hh:16 * hh + 16, hh, :],
                              q_sb[16 * hh:16 * hh + 16, :])
            nc.sync.dma_start(qm1[64 + 16 * hh:80 + 16 * hh, hh, :],
                              q_sb[64 + 16 * hh:80 + 16 * hh, :])

        # vT = x^T @ vwT, stored [112, 7, 8, 65] with ones col
        vt = vt_ab[b % 2]
        for c in range(NCH):
            ps = pp.tile([CHK, DH], F32, tag="proj")
            for j in range(3):
                nc.tensor.matmul(ps[:], x[:, j, CHK * c:CHK * c + CHK],
                                 vwT[:, j, :], start=(j == 0), stop=(j == 2))
            nc.scalar.activation(
                vt[:, c, :, 0:VD],
                ps.rearrange("p (h d) -> p h d", h=NH), AF.Copy)

        # k = kw @ x + kb, split into the two kcomb halves
        for nhalf in range(2):
            ps = pp.tile([128, 392], F32, tag="proj")
            for j in range(3):
                nc.tensor.matmul(ps[:], kwT[:, j, :],
                                 x[:, j, 392 * nhalf:392 * nhalf + 392],
                                 start=(j == 0), stop=(j == 2))
            sl = slice(392 * nhalf, 392 * nhalf + 392)
            # two engines so both halves retire in parallel (frees the
            # proj-PSUM ring buffer sooner); GPSIMD can't read PSUM, so the
            # second half rides the ACT engine as a biased copy
            nc.vector.tensor_scalar_add(kc0[0:64, sl], ps[0:64, :],
                                        bias[0:64, 0:1])
            nc.vector.tensor_scalar_add(kc1[64:128, sl], ps[64:128, :],
                                        bias[64:128, 0:1])

    def vc_chunk(b, c):
        # v channel-major via PE transpose of vT for position chunk c;
        # vc[128ch, padded planes] = vt[pos, ch]^T, tile t = heads 2t,2t+1.
        # Each 112-pos chunk is 8 rows of 14, possibly crossing one plane
        # boundary; pads stay zero (no bias folded into vc). One matmul per
        # head: the [112, 2, 64] slice of vt has two unmergeable free dims
        # (head stride 65), which the BIR verifier rejects as an operand.
        vt = vt_ab[b % 2]
        vc = vc_ab[b % 2]
        for t in range(4):
            ps = pp.tile([128, 256], F32, tag="proj")
            nc.tensor.matmul(ps[0:VD, 0:CHK],
                             vt[:, c, 2 * t, 0:VD],
                             ident[:], start=True, stop=True)
            nc.tensor.matmul(ps[VD:128, 0:CHK],
                             vt[:, c, 2 * t + 1, 0:VD],
                             ident[:], start=True, stop=True)
            start = CHK * c
            s0, r0 = start // 196, (start % 196) // 14
            rows0 = min(8, 14 - r0)
            src = ps[:, 0:CHK].rearrange("p (r w) -> p r w", w=14)
            dst0 = vc[:, t, 225 * s0:225 * s0 + 225].rearrange(
                "p (r w) -> p r w", w=15)[:, 1 + r0:1 + r0 + rows0, 1:15]
            nc.vector.tensor_copy(dst0, src[:, 0:rows0, :])
            if rows0 < 8:
                dst1 = vc[:, t, 225 * (s0 + 1):225 * (s0 + 1) + 225
                          ].rearrange("p (r w) -> p r w",
                                      w=15)[:, 1:9 - rows0, 1:15]
                nc.vector.tensor_copy(dst1, src[:, rows0:8, :])
        return vc

    def attn_chunks(b, hg, o_of, filler=None):
        """filler(c): emits independent PE work after chunk c's PV matmuls
        so the in-order PE queue has something during the ACT-paced gaps."""
        vt = vt_ab[b % 2]
        kc = kcomb[(b % 2, hg)]
        qq = qmu[(b % 2, hg)]
        for c in range(NCH):
            s_ps = sp.tile([CHK, 2, 512], F32, tag="s")
            for hh in range(4):
                sl = s_ps[:, hh // 2, 196 * (hh % 2):196 * (hh % 2) + 196]
                nc.tensor.matmul(sl, kc[:, CHK * c:CHK * c + CHK],
                                 qq[:, hh, :],
                                 start=(hh % 2 == 0), stop=(hh % 2 == 1))
            es = ep.tile([CHK, 4, N2], BF16, tag="es")
            nc.scalar.activation(
                es.rearrange("p a q -> p (a q)").rearrange(
                    "p (a q) -> p a q", a=2),
                s_ps[:, :, 0:392],
                AF.Exp)
            for hh in range(4):
                h = 4 * hg + hh
                ot, osl = o_of[hh]
                nc.tensor.matmul(
                    ot[:, osl, 0:N2],
                    vt[:, c, h, :], es[:, hh, :],
                    start=(c == 0 and hh % 2 == 0),
                    stop=(c == NCH - 1 and hh % 2 == 1))
            if filler is not None:
                filler(c)

    def norm_recips(rec, hg, o_of):
        o_pa, o_pb = o_of[0][0], o_of[2][0]
        with nc.allow_low_precision(reason="softmax recip in bf16"):
            nc.vector.reciprocal(
                rec[VD:VD + 1, 4 * hg:4 * hg + 2, :],
                o_pa[VD:VD + 1, :, 0:N2])
            nc.vector.reciprocal(
                rec[VD:VD + 1, 4 * hg + 2:4 * hg + 4, :],
                o_pb[VD:VD + 1, :, 0:N2])

    def norm_apply(rec, bcs, ts2, todd, hg, o_of):
        # broadcast 1/den from partition VD across the 64 v-dim partitions
        # (lhsT/rhs share partition base VD), stage to SBUF, then scale.
        for u in range(2):
            bc = pp.tile([VD, 512], F32, tag="proj")
            nc.tensor.matmul(
                bc[:, 0:392], ones[VD:VD + 1, 0:VD],
                rec[VD:VD + 1, 4 * hg + 2 * u:4 * hg + 2 * u + 2,
                    :].rearrange("p u q -> p (u q)"),
                start=True, stop=True)
            nc.vector.tensor_copy(
                bcs[:, 4 * hg + 2 * u:4 * hg + 2 * u + 2, :],
                bc[:, 0:392].rearrange("p (u q) -> p u q", u=2))
        for hh in range(4):
            h = 4 * hg + hh
            ot, osl = o_of[hh]
            dst = (ts2[0:VD, h // 2, :] if h % 2 == 0
                   else todd[:, h // 2, :])
            nc.vector.tensor_tensor(out=dst, in0=ot[0:VD, osl, 0:N2],
                                    in1=bcs[:, h, :], op=ALU.mult)

    def vl_tile(vc, vl, t):
        # v_local dwconv for channel tile t + all folded biases
        ps = pp.tile([128, 256], F32, tag="proj")
        for ti in range(9):
            a, bb = divmod(ti, 3)
            pr, pc = tap_geom(a), tap_geom(bb)
            off = (15 if a else 0) + (1 if bb else 0)
            nc.tensor.matmul(
                ps[:, 0:209], vd[:, 9 * t + ti, :],
                vc[:, t, 225 * (2 * pr + pc) + off:
                   225 * (2 * pr + pc) + off + 209],
                start=(ti == 0), stop=(ti == 8))
        if t % 2 == 0:
            nc.vector.tensor_scalar_add(
                vl[:, t, :].rearrange("p (r c) -> p r c", r=14),
                ps[:, 0:210].rearrange("p (r c) -> p r c", c=15)[:, :, 0:14],
                bias[:, 2 + t:3 + t])
        else:
            nc.scalar.activation(
                vl[:, t, :].rearrange("p (r c) -> p r c", r=14),
                ps[:, 0:210].rearrange("p (r c) -> p r c", c=15)[:, :, 0:14],
                AF.Identity, bias=bias[:, 2 + t:3 + t])

    head(0)
    for b in range(B_LOC):
        rec = sb.tile([VD + 1, NH, N2], BF16, tag="rec")
        bcs = sb.tile([VD, NH, N2], F32, tag="bcs")
        ts2 = sb.tile([128, 4, N2], BF16, tag="ts2")
        todd = sb.tile([VD, 4, N2], BF16, tag="todd")
        o_hg = []
        for hg in range(2):
            o_pa = op.tile([VD + 1, 2, 256], F32, tag="o")
            o_pb = op.tile([VD + 1, 2, 256], F32, tag="o")
            o_hg.append([(o_pa, 0), (o_pa, 1), (o_pb, 0), (o_pb, 1)])
        # the vc transposes ride along inside hg0's ACT-paced chunk loop
        # (chunk c of vc needs only chunk c of vt), and the vl dwconv tiles
        # ride inside hg1's (they need all of vc, complete by then)
        vc = vc_ab[b % 2]
        vl = sb.tile([128, 4, N2], BF16, tag="vl")
        attn_chunks(b, 0, o_hg[0], filler=lambda c: vc_chunk(b, c))
        norm_recips(rec, 0, o_hg[0])
        norm_apply(rec, bcs, ts2, todd, 0, o_hg[0])
        attn_chunks(b, 1, o_hg[1],
                    filler=lambda c: vl_tile(vc, vl, c) if c < 4 else None)
        norm_recips(rec, 1, o_hg[1])
        norm_apply(rec, bcs, ts2, todd, 1, o_hg[1])
        if b + 1 < B_LOC:
            head(b + 1)         # next batch's head hides this batch's tail

        nc.sync.dma_start(ts2[VD:128, :, :], todd[:])
        r_sb = sb.tile([128, 4, N2], BF16, tag="r")
        nc.vector.tensor_tensor(out=r_sb[:], in0=ts2[:], in1=vl[:],
                                op=ALU.add)
        nc.vector.tensor_scalar_max(r_sb[:], r_sb[:], 0.0)

        # output projection
        o_sb = sb.tile([128, 3, N2], BF16, tag="os")
        for m in range(3):
            ps = pp.tile([128, N2], F32, tag="proj")
            for j in range(4):
                nc.tensor.matmul(ps[:], pwT[:, j, 128 * m:128 * m + 128],
                                 r_sb[:, j, :], start=(j == 0), stop=(j == 3))
            nc.vector.tensor_scalar_add(o_sb[:, m, :], ps[:],
                                        bias[:, 6 + m:7 + m])
        nc.sync.dma_start(out_d[b].rearrange("(m p) q -> p m q", p=128),
                          o_sb[:])
    ctx.close()


# ----------------------------------------------------------------------------
# persistent runner (AOT fast-dispatch PJRT executable + device consts)
# ----------------------------------------------------------------------------
class _State:
    pass


_STATE = None
_LAST_EXEC_NS = None


def last_exec_ns():
    return _LAST_EXEC_NS


def _ensure_state():
    global _STATE
    if _STATE is not None:
        return _STATE
    st = _State()
    nc = build_program()
    install_neuronx_cc_hook()
    partition_name = (nc.partition_id_tensor.name
                      if nc.partition_id_tensor else None)
    in_names, out_names, out_avals = [], [], []
    for alloc in nc.m.functions[0].allocations:
        if not isinstance(alloc, mybir.MemoryLocationSet):
            continue
        name = alloc.memorylocations[0].name
        if alloc.kind == "ExternalInput":
            if name != partition_name:
                in_names.append(name)
        elif alloc.kind == "ExternalOutput":
            out_names.append(name)
            out_avals.append(jax.core.ShapedArray(
                tuple(alloc.tensor_shape), mybir.dt.np(alloc.dtype)))
    assert in_names[0] == "x" and out_names == ["out"]
    all_in_names = list(in_names) + list(out_names)
    if partition_name is not None:
        all_in_names.append(partition_name)

    def _body(*args):
        operands = list(args)
        if partition_name is not None:
            operands.append(partition_id_tensor())
        outs = _bass_exec_p.bind(
            *operands, out_avals=tuple(out_avals),
            in_names=tuple(all_in_names), out_names=tuple(out_names),
            lowering_input_output_aliases=(),
            sim_require_finite=True, sim_require_nnan=True, nc=nc)
        return tuple(outs)

    devices = jax.devices()[:N_CORES]
    mesh = Mesh(np.asarray(devices), ("core",))
    sh = NamedSharding(mesh, PartitionSpec("core"))
    n_in = len(in_names)
    donate = (n_in,)  # the recycled output buffer
    in_specs = (PartitionSpec("core"),) * (n_in + 1)
    out_specs = (PartitionSpec("core"),)

    # global (concat-over-cores) shapes for every operand
    per_core_shapes = {}
    for alloc in nc.m.functions[0].allocations:
        if not isinstance(alloc, mybir.MemoryLocationSet):
            continue
        name = alloc.memorylocations[0].name
        if name in in_names or name in out_names:
            per_core_shapes[name] = (tuple(alloc.tensor_shape),
                                     np.dtype(mybir.dt.np(alloc.dtype)))

    def gshape(name):
        s, d = per_core_shapes[name]
        return jax.ShapeDtypeStruct((N_CORES * s[0], *s[1:]), d, sharding=sh)

    arg_specs = [gshape(nm) for nm in in_names] + [gshape("out")]

    def _compile():
        jitted = jax.jit(
            shard_map(_body, mesh=mesh, in_specs=in_specs,
                      out_specs=out_specs, check_rep=False),
            donate_argnums=donate, keep_unused=True)
        return jitted.lower(*arg_specs).compile()

    st.fn = fast_dispatch_compile(_compile)
    st.sh = sh
    st.in_names = in_names
    st.out_shape = per_core_shapes["out"]
    st.consts_key = None
    st.consts_dev = None
    st.prev_out = None
    _STATE = st
    return st


def _ensure_consts(st, inputs):
    key = [np.asarray(inputs[k]) for k in _WEIGHT_NAMES]
    if st.consts_key is not None and all(
            np.array_equal(a, b) for a, b in zip(st.consts_key, key)):
        return
    consts = _prep_consts(inputs)
    dev = []
    for nm in st.in_names[1:]:
        a = consts[nm]
        g = np.broadcast_to(a, (N_CORES, *a.shape)).reshape(
            N_CORES * a.shape[0], *a.shape[1:])
        dev.append(jax.device_put(np.ascontiguousarray(g), st.sh))
    jax.block_until_ready(dev)
    st.consts_dev = dev
    st.consts_key = key


def _pack_x(x4):
    """[64,C,28,28] f32 -> bf16 [64,C,784] in polyphase plane order."""
    xb = x4.astype(bf)
    x6 = xb.reshape(B, C, 14, 2, 14, 2)
    xr = x6.transpose(0, 1, 3, 5, 2, 4)      # [B,C,pr,pc,r,c]
    return np.ascontiguousarray(xr).reshape(B, C, N)


def _fresh_outbuf(st):
    s, d = st.out_shape
    return jax.device_put(np.zeros((N_CORES * s[0], *s[1:]), d), st.sh)


def kernel(**inputs):
    global _LAST_EXEC_NS
    st = _ensure_state()
    _ensure_consts(st, inputs)
    xg = _pack_x(np.asarray(inputs["x"], np.float32))
    x_dev = jax.device_put(xg, st.sh)
    ob = st.prev_out if st.prev_out is not None else _fresh_outbuf(st)
    jax.block_until_ready([x_dev, ob])
    t0 = time.perf_counter()
    out, = st.fn(x_dev, *st.consts_dev, ob)
    out.block_until_ready()
    t1 = time.perf_counter()
    _LAST_EXEC_NS = int((t1 - t0) * 1e9)
    host = np.asarray(out)                    # [64, OUT, N2] bf16
    st.prev_out = out
    st.last_x = x_dev
    return host.astype(np.float32).reshape(B, OUT, H2, W2)


def exec_pipeline(n):
    """Run the compiled NEFF n times back-to-back on device (inputs staged by
    the last kernel() call), one sync at the end. Output buffers are donation-
    chained: execute i reuses the buffer produced by execute i-2, so the chain
    runs entirely on device. Returns wall seconds for the n executes + sync."""
    st = _STATE
    assert st is not None and st.prev_out is not None, "call kernel() first"
    if getattr(st, "spare", None) is None:
        st.spare = _fresh_outbuf(st)
    bufs = [st.prev_out, st.spare]
    jax.block_until_ready([st.last_x, *bufs])
    t0 = time.perf_counter()
    for i in range(n):
        o, = st.fn(st.last_x, *st.consts_dev, bufs[i])
        bufs.append(o)
    bufs[-1].block_until_ready()
    t1 = time.perf_counter()
    # donated: bufs[0..n-1]; still live: bufs[n], bufs[n+1] (the last two)
    st.prev_out, st.spare = bufs[-1], bufs[-2]
    return t1 - t0



# revision 2
# speedup vs baseline: 1.1090x; 1.1090x over previous
"""Trainium2 Bass kernel for nn_Attention4DDownsample.

Sharding: data-parallel over batch B=64 across 8 cores (8 batches/core).
All parameters replicated. Device program per batch (software-pipelined
across batches: head(b+1) is emitted inside tail(b) so the in-order PE
queue always has independent work during the DVE/DMA tail chains):
  head: x load + polyphase padded planes (built on device),
        q = dwconv+pool+1x1 fused as 27 PSUM-accumulating matmuls,
        k = fold_bn(k_w) @ x, vT = x^T @ v_w^T (+ ones col, per-chunk)
  per head-group hg (4 heads) and k-chunk c (7 x 112):
    S^T[c] = k^T q (K=16) ++ bias via rank-16 U@Mr^T matmul (PSUM accum)
    P[c]   = exp(S^T[c])   (ACT, bf16 out)
    o     += vT[c]^T @ P[c] (PSUM accum; row 64 = softmax denominator)
    + filler PE work riding the ACT-paced loop: hg0 carries the vc
      channel-major transposes of vT (identity matmuls), hg1 the v_local
      dwconv tiles (9 diag-matmuls each)
  normalize: 1/den on partition 64, broadcast via a matmul whose lhsT and
  rhs both sit at partition base 64 (no partition-shift DMA), o *= bcast
  R = relu(o_n + v_local); out = fold(p_w) @ R + b  (bf16)

Host side keeps a persistent AOT-compiled (fast-dispatch) PJRT executable,
device-resident replicated constants (re-uploaded only when the weight
inputs actually change), and recycles the previous call's device output as
the next call's donated output buffer, so a steady-state call ships only
the activation tensor x and reads back the bf16 output.
"""

import os
import sys
import time

for p in ("/opt/trn_rl_repo",):
    if p not in sys.path and os.path.isdir(p):
        sys.path.insert(0, p)
os.environ.setdefault("MYCRO_LOCAL_CACHE", "1")

import numpy as np
import ml_dtypes

import jax
from jax.sharding import Mesh, PartitionSpec, NamedSharding
from jax.experimental.shard_map import shard_map

import concourse.bass as bass
import concourse.mybir as mybir
import concourse.tile as tile
from concourse import bacc
from concourse.bass2jax import (
    _bass_exec_p,
    install_neuronx_cc_hook,
    partition_id_tensor,
    fast_dispatch_compile,
)

BF16 = mybir.dt.bfloat16
F32 = mybir.dt.float32
AF = mybir.ActivationFunctionType
ALU = mybir.AluOpType

N_CORES = 8
B = 64             # full batch
B_LOC = 8          # batches per core
C = 384            # input channels
H = W = 28
N = H * W          # 784 key positions
H2 = W2 = 14
N2 = H2 * W2       # 196 query positions
NH = 8             # heads
KD = 16            # head dim (qk)
DH = 512           # v channels
VD = 64            # v head dim
OUT = 384          # output channels
NCH = 7            # k-position chunks
CHK = 112          # chunk size (7*112 = 784)

bf = ml_dtypes.bfloat16


# ----------------------------------------------------------------------------
# host-side constant prep (bicubic matrices are shape-deterministic)
# ----------------------------------------------------------------------------
_A_CUBIC = -0.75


def _cubic_kernel(x):
    A = _A_CUBIC
    x = np.abs(x)
    return np.where(
        x <= 1.0,
        ((A + 2.0) * x - (A + 3.0)) * x * x + 1.0,
        np.where(x < 2.0, ((A * x - 5.0 * A) * x + 8.0 * A) * x - 4.0 * A, 0.0),
    ).astype(np.float32)


def _bicubic_matrix(out_size, in_size):
    i = np.arange(out_size)
    s = (i + 0.5) * in_size / out_size - 0.5
    i0 = np.floor(s).astype(np.int64)
    t = s - i0
    M = np.zeros((out_size, in_size), np.float32)
    for o in (-1, 0, 1, 2):
        idx = np.clip(i0 + o, 0, in_size - 1)
        np.add.at(M, (i, idx), _cubic_kernel(t - o))
    return M


_WEIGHT_NAMES = (
    "q_local_w", "q_local_b", "q_proj_w", "q_proj_b", "q_bn_s", "q_bn_b",
    "k_w", "k_b", "k_bn_s", "k_bn_b", "v_w", "v_b", "v_bn_s", "v_bn_b",
    "vl_w", "vl_b", "vl_bn_s", "vl_bn_b", "p_w", "p_b", "p_bn_s", "p_bn_b",
    "ab_table", "bias_idxs",
)


def _prep_consts(inputs):
    """Fold BNs/scales into weights, build transposed/bias/diag tensors."""
    f = {k: np.asarray(inputs[k], np.float32) for k in _WEIGHT_NAMES
         if k != "bias_idxs"}
    bias_idxs = np.asarray(inputs["bias_idxs"])

    scale = KD ** -0.5
    # q: q = scale * bn(q_proj @ (dwconv_aug(x) + q_local_b))
    qw = (f["q_bn_s"][:, None] * f["q_proj_w"]) * scale       # [128, 384]
    qb = scale * (f["q_bn_s"] * f["q_proj_b"] + f["q_bn_b"])  # [128]
    qb = qb + qw @ f["q_local_b"]                              # fold dw bias
    kw = f["k_bn_s"][:, None] * f["k_w"]
    kb = f["k_bn_s"] * f["k_b"] + f["k_bn_b"]
    vw = f["v_bn_s"][:, None] * f["v_w"]                       # [512, 384]
    vbeta = f["v_bn_s"] * f["v_b"] + f["v_bn_b"]               # [512]
    # v_local = bn_vl(dwconv(v0 + vbeta, vl_w) + vl_b); o gets +vbeta after
    # normalization. Fold everything constant into one per-channel bias.
    vlw = f["vl_bn_s"][:, None, None] * f["vl_w"][:, 0]        # [512, 3, 3]
    tapsum = f["vl_w"][:, 0].sum(axis=(1, 2))                  # [512]
    vlb = (f["vl_bn_s"] * (vbeta * tapsum + f["vl_b"]) + f["vl_bn_b"]
           + vbeta)                                            # [512]
    pw = f["p_bn_s"][:, None] * f["p_w"]                       # [384, 512]
    pb = f["p_bn_s"] * f["p_b"] + f["p_bn_b"]                  # [384]

    # q dwconv weights with the avgpool folded in as +1 on the center tap
    qlw = f["q_local_w"][:, 0].copy()                          # [384, 3, 3]
    qlw[:, 1, 1] += 1.0

    # attention bias, rank-16 factorization: bias^T_h = U_h @ Mr^T,
    # U_h = Mc @ ab_h^T  [784, 16]
    ab = f["ab_table"][:, bias_idxs]                           # [8, 16, 49]
    Mr = _bicubic_matrix(N2, 16)                               # [196, 16]
    Mc = _bicubic_matrix(N, 49)                                # [784, 49]
    # x is shipped phase-reordered (4 stride-2 planes concatenated); the
    # attention is permutation-invariant over key positions as long as the
    # bias factor U is permuted identically.
    perm = []
    for pr in range(2):
        for pc in range(2):
            for r in range(14):
                for cc2 in range(14):
                    perm.append((2 * r + pr) * W + (2 * cc2 + pc))
    perm = np.asarray(perm)
    UT = np.zeros((128, N), np.float32)                        # rows 16h+j
    for h in range(NH):
        U = (Mc @ ab[h].T)[perm]                               # [784, 16]
        UT[16 * h:16 * h + 16] = U.T

    # Combined S^T lhsT layout: kcomb_hg = [k rows | U rows] where for
    # hg=0: rows 0-63 = k heads 0-3, rows 64-127 = U heads 0-3; for hg=1
    # mirrored (U heads 4-7 in rows 0-63, k heads 4-7 in rows 64-127) so
    # the dynamic k half lands on its natural partition range. The rhs
    # qmu_hg[h] masks both q (head rows) and Mr^T (bias rank rows).
    qmu_init = np.zeros((2, 128, 4, N2), np.float32)
    for hh in range(4):
        qmu_init[0, 64 + 16 * hh:80 + 16 * hh, hh] = Mr.T   # bias rows hg0
        qmu_init[1, 16 * hh:16 * hh + 16, hh] = Mr.T        # bias rows hg1

    # q dwconv folded into the q projection: for input-channel block j and
    # tap (a,b), lhsT[c, o] = qwT[c, o] * qlw[c, a, b] -- the 27 matmuls
    # accumulate q = sum_tap qw @ (w_tap * x_shift_tap) directly in PSUM.
    qdw = np.zeros((3, 9, 128, 128), np.float32)
    qwT_full = qw.T                                            # [384, 128]
    for j in range(3):
        for a in range(3):
            for b in range(3):
                blk = slice(128 * j, 128 * j + 128)
                qdw[j, 3 * a + b] = (qwT_full[blk]
                                     * qlw[blk, a, b][:, None])
    # diag matrices for the v_local depthwise conv
    vd = np.zeros((4, 9, 128, 128), np.float32)
    for t in range(4):
        for a in range(3):
            for b in range(3):
                np.fill_diagonal(vd[t, 3 * a + b],
                                 vlw[128 * t:128 * t + 128, a, b])

    # per-partition bias pack [128, 9]:
    # col 0: kb, 1: qb, 2-5: vlb (4 ptiles), 6-8: pb (3 ptiles)
    bias_pack = np.zeros((128, 9), np.float32)
    bias_pack[:, 0] = kb
    bias_pack[:, 1] = qb
    for t in range(4):
        bias_pack[:, 2 + t] = vlb[128 * t:128 * t + 128]
    for m in range(3):
        bias_pack[:, 6 + m] = pb[128 * m:128 * m + 128]

    consts = {
        "qmu_init": qmu_init.astype(bf),                       # [2,128,4,196]
        "kwT": np.ascontiguousarray(kw.T).astype(bf),          # [384, 128]
        "vwT": np.ascontiguousarray(vw.T).astype(bf),          # [384, 512]
        "pwT": np.ascontiguousarray(pw.T).astype(bf),          # [512, 384]
        "ut": UT.astype(bf),                                   # [128, 784]
        "qd": qdw.astype(bf),                                  # [3,9,128,128]
        "vd": vd.astype(bf),                                   # [4,9,128,128]
        "bias_pack": bias_pack,                                # [128, 9] f32
    }
    return consts


# ----------------------------------------------------------------------------
# device program
# ----------------------------------------------------------------------------
def build_program():
    nc = bacc.Bacc()
    x_d = nc.declare_dram_parameter("x", [B_LOC, 128, 3, N], BF16,
                                    isOutput=False)
    kwT_d = nc.declare_dram_parameter("kwT", [C, 128], BF16, isOutput=False)
    vwT_d = nc.declare_dram_parameter("vwT", [C, DH], BF16, isOutput=False)
    pwT_d = nc.declare_dram_parameter("pwT", [DH, OUT], BF16, isOutput=False)
    ut_d = nc.declare_dram_parameter("ut", [128, N], BF16, isOutput=False)
    qmu_d = nc.declare_dram_parameter("qmu_init", [2, 128, 4, N2], BF16,
                                      isOutput=False)
    qd_d = nc.declare_dram_parameter("qd", [3, 9, 128, 128], BF16,
                                     isOutput=False)
    vd_d = nc.declare_dram_parameter("vd", [4, 9, 128, 128], BF16,
                                     isOutput=False)
    bias_d = nc.declare_dram_parameter("bias_pack", [128, 9], F32,
                                       isOutput=False)
    out_d = nc.declare_dram_parameter("out", [B_LOC, 128, 3, N2], BF16,
                                      isOutput=True)

    with tile.TileContext(nc) as tc:
        _emit(nc, tc, x_d, kwT_d, vwT_d, pwT_d, ut_d,
              qmu_d, qd_d, vd_d, bias_d, out_d)
    nc.finalize()
    return nc


def _emit(nc, tc, x_d, kwT_d, vwT_d, pwT_d, ut_d,
          qmu_d, qd_d, vd_d, bias_d, out_d):
    from contextlib import ExitStack
    ctx = ExitStack()
    cp = ctx.enter_context(tc.tile_pool(name="consts", bufs=1))
    xp = ctx.enter_context(tc.tile_pool(name="xp", bufs=4))
    sb = ctx.enter_context(tc.tile_pool(name="sb", bufs=3))
    ep = ctx.enter_context(tc.tile_pool(name="ep", bufs=6))
    pp = ctx.enter_context(tc.tile_pool(name="pp", bufs=2, space="PSUM"))
    sp = ctx.enter_context(tc.tile_pool(name="sp", bufs=3, space="PSUM"))
    op = ctx.enter_context(tc.tile_pool(name="op", bufs=2, space="PSUM"))
    vp = ctx.enter_context(tc.tile_pool(name="vp", bufs=1, space="PSUM"))

    # ---- load constants (head(0)-critical tensors first; DMAs spread over
    # engine queues so descriptor generation runs in parallel) ---------------
    qd = cp.tile([128, 27, 128], BF16)
    nc.sync.dma_start(qd[:], qd_d.rearrange("t k p m -> p (t k) m"))
    vwT = cp.tile([128, 3, DH], BF16)
    nc.scalar.dma_start(vwT[:], vwT_d.rearrange("(j p) m -> p j m", p=128))
    kwT = cp.tile([128, 3, 128], BF16)
    nc.gpsimd.dma_start(kwT[:], kwT_d.rearrange("(j p) m -> p j m", p=128))
    bias = cp.tile([128, 9], F32)
    nc.gpsimd.dma_start(bias[:], bias_d[:])
    # persistent (batch-parity double buffered) combined lhsT and masked rhs
    kcomb = {}
    qmu = {}
    for par in range(2):
        for hg in range(2):
            kt = cp.tile([128, N], BF16, tag=f"kc{par}{hg}")
            eng = nc.scalar if (par + hg) % 2 == 0 else nc.gpsimd
            # static U half: hg0 -> rows 64-127, hg1 -> rows 0-63
            if hg == 0:
                eng.dma_start(kt[64:128, :], ut_d[0:64, :])
            else:
                eng.dma_start(kt[0:64, :], ut_d[64:128, :])
            kcomb[(par, hg)] = kt
            qt = cp.tile([128, 4, N2], BF16, tag=f"qm{par}{hg}")
            eng2 = nc.gpsimd if (par + hg) % 2 == 0 else nc.scalar
            eng2.dma_start(qt[:], qmu_d[hg])
            qmu[(par, hg)] = qt
    vd = cp.tile([128, 36, 128], BF16)
    nc.scalar.dma_start(vd[:], vd_d.rearrange("t k p m -> p (t k) m"))
    pwT = cp.tile([128, 4, OUT], BF16)
    nc.scalar.dma_start(pwT[:], pwT_d.rearrange("(j p) m -> p j m", p=128))
    ones = cp.tile([128, VD], BF16)
    nc.vector.memset(ones[:], 1.0)
    vt_ab = []
    for i in range(2):
        t = cp.tile([CHK, NCH, NH, VD + 1], BF16, tag=f"vt{i}")
        for c in range(NCH):
            nc.vector.memset(t[:, c, :, VD:VD + 1], 1.0)
        vt_ab.append(t)
    # polyphase padded planes of x, built on device (pads zeroed once):
    # plane (pr,pc) of chunk j: 15x15, pad row0/col0 zero, data
    # P[r+1,c+1] = x[2r+pr, 2c+pc]. A 3x3/stride-2 tap is then ONE
    # contiguous 209-elem run at offset 15*(dr+1)+(dc+1) (dr,dc in
    # {-1,0}) -- the pad col absorbs the row wrap with zeros.
    xph_ab = []
    for i in range(2):
        t = cp.tile([128, 12, 225], BF16, tag=f"xph{i}")
        nc.gpsimd.memset(t[:], 0.0)
        xph_ab.append(t)
    # vc planes likewise live in persistent parity buffers with zero pads
    vc_ab = []
    for i in range(2):
        t = cp.tile([128, 4, 900], BF16, tag=f"vc{i}")
        nc.gpsimd.memset(t[:], 0.0)
        vc_ab.append(t)

    def tap_geom(a):
        # returns the phase index component for row or col tap index
        return 1 if a == 0 else 0 if a == 1 else 1

    x_tiles = {}

    def head_load(b):
        """x load + polyphase scatter for batch b (issued early so the
        compute half never waits on the DMA + gpsimd scatter chain)."""
        x = xp.tile([128, 3, N], BF16, tag="x")
        nc.sync.dma_start(x[:], x_d[b])
        xph = xph_ab[b % 2]
        # scatter the 12 14x14 planes into the zero-padded 15x15 layout
        nc.gpsimd.tensor_copy(
            xph.rearrange("p g (r c) -> p g r c", r=15)[:, :, 1:15, 1:15],
            x.rearrange("p j (s r c) -> p (j s) r c", s=4, r=14))
        x_tiles[b] = x

    def head_compute(b):
        """q dwconv/proj + k + vT for batch b."""
        x = x_tiles.pop(b)
        xph = xph_ab[b % 2]
        kc0 = kcomb[(b % 2, 0)]
        kc1 = kcomb[(b % 2, 1)]
        qm0 = qmu[(b % 2, 0)]
        qm1 = qmu[(b % 2, 1)]
        # q dwconv with the 1x1 projection folded in: 27 matmuls accumulate
        # q = sum_{j,tap} (qw_j * w_tap) @ xph_shift straight into PSUM
        q_sb = sb.tile([128, N2], BF16, tag="q")
        ps = pp.tile([128, 256], F32, tag="proj")
        for j in range(3):
            for ti in range(9):
                a, bb = divmod(ti, 3)
                pr, pc = tap_geom(a), tap_geom(bb)
                off = (15 if a else 0) + (1 if bb else 0)
                nc.tensor.matmul(
                    ps[:, 0:209], qd[:, 9 * j + ti, :],
                    xph[:, 4 * j + 2 * pr + pc, off:off + 209],
                    start=(j == 0 and ti == 0), stop=(j == 2 and ti == 8))
        nc.vector.tensor_scalar_add(
            q_sb.rearrange("p (r c) -> p r c", r=14),
            ps[:, 0:210].rearrange("p (r c) -> p r c", c=15)[:, :, 0:14],
            bias[:, 1:2])
        for hh in range(4):
            nc.sync.dma_start(qm0[16 * hh:16 * hh + 16, hh, :],
                              q_sb[16 * hh:16 * hh + 16, :])
            nc.sync.dma_start(qm1[64 + 16 * hh:80 + 16 * hh, hh, :],
                              q_sb[64 + 16 * hh:80 + 16 * hh, :])

        # vT = x^T @ vwT, stored [112, 7, 8, 65] with ones col (PV lhsT)
        vt = vt_ab[b % 2]
        for c in range(NCH):
            ps = pp.tile([CHK, DH], F32, tag="proj")
            for j in range(3):
                nc.tensor.matmul(ps[:], x[:, j, CHK * c:CHK * c + CHK],
                                 vwT[:, j, :], start=(j == 0), stop=(j == 2))
            nc.scalar.activation(
                vt[:, c, :, 0:VD],
                ps.rearrange("p (h d) -> p h d", h=NH), AF.Copy)

        # k = kw @ x + kb, split into the two kcomb halves
        for nhalf in range(2):
            ps = pp.tile([128, 392], F32, tag="proj")
            for j in range(3):
                nc.tensor.matmul(ps[:], kwT[:, j, :],
                                 x[:, j, 392 * nhalf:392 * nhalf + 392],
                                 start=(j == 0), stop=(j == 2))
            sl = slice(392 * nhalf, 392 * nhalf + 392)
            nc.vector.tensor_scalar_add(kc0[0:64, sl], ps[0:64, :],
                                        bias[0:64, 0:1])
            nc.vector.tensor_scalar_add(kc1[64:128, sl], ps[64:128, :],
                                        bias[64:128, 0:1])

        # v channel-major directly from x (feeds the v_local dwconv): per
        # 128-ch block and plane pair, out[ch, pos] scattered into the
        # zero-padded 15x15 planes with one copy per half. Replaces the
        # old vt->PE-transpose->scatter chain inside the attention loop.
        vc = vc_ab[b % 2]
        for t in range(4):
            for half in range(2):
                ps = pp.tile([128, 392], F32, tag="proj")
                for j in range(3):
                    nc.tensor.matmul(
                        ps[:], vwT[:, j, 128 * t:128 * t + 128],
                        x[:, j, 392 * half:392 * half + 392],
                        start=(j == 0), stop=(j == 2))
                dst = vc[:, t, :].rearrange(
                    "p (s r w) -> p s r w", s=4, w=15)[
                    :, 2 * half:2 * half + 2, 1:15, 1:15]
                nc.vector.tensor_copy(
                    dst, ps.rearrange("p (s r w) -> p s r w", s=2, w=14))

    def s_chunk(b, hg, c):
        """S^T matmuls for chunk c + exp; returns the es tile for pv_chunk.
        Emitted one chunk AHEAD of its pv_chunk so the in-order PE queue has
        independent work while the ACT exp (plus two semaphore hops) runs."""
        kc = kcomb[(b % 2, hg)]
        qq = qmu[(b % 2, hg)]
        es = ep.tile([CHK, 4, N2], BF16, tag="es")
        for u in range(2):
            # two heads per matmul: their q/Mr rows occupy disjoint
            # contraction rows, their outputs adjacent 196-col blocks;
            # single-bank PSUM tile + exp per half so ACT starts as soon
            # as its half is done
            s_ps = sp.tile([CHK, 512], F32, tag="s")
            nc.tensor.matmul(
                s_ps[:, 0:392], kc[:, CHK * c:CHK * c + CHK],
                qq[:, 2 * u:2 * u + 2, :].rearrange("p a q -> p (a q)"),
                start=True, stop=True)
            nc.scalar.activation(
                es[:, 2 * u:2 * u + 2, :], s_ps[:, 0:392].rearrange(
                    "p (a q) -> p a q", a=2),
                AF.Exp)
        return es

    def pv_chunk(b, hg, c, es, o_of):
        vt = vt_ab[b % 2]
        for hh in range(4):
            h = 4 * hg + hh
            ot, osl = o_of[hh]
            nc.tensor.matmul(
                ot[:, osl, 0:N2],
                vt[:, c, h, :], es[:, hh, :],
                start=(c == 0 and hh % 2 == 0),
                stop=(c == NCH - 1 and hh % 2 == 1))

    def norm_recips(rec, hg, o_of):
        o_pa, o_pb = o_of[0][0], o_of[2][0]
        with nc.allow_low_precision(reason="softmax recip in bf16"):
            nc.vector.reciprocal(
                rec[VD:VD + 1, 4 * hg:4 * hg + 2, :],
                o_pa[VD:VD + 1, :, 0:N2])
            nc.vector.reciprocal(
                rec[VD:VD + 1, 4 * hg + 2:4 * hg + 4, :],
                o_pb[VD:VD + 1, :, 0:N2])

    def norm_bcast(rec, bcs, hg):
        # broadcast 1/den from partition VD across the 64 v-dim partitions
        # (lhsT/rhs share partition base VD), stage to SBUF
        for u in range(2):
            bc = pp.tile([VD, 512], F32, tag="proj")
            nc.tensor.matmul(
                bc[:, 0:392], ones[VD:VD + 1, 0:VD],
                rec[VD:VD + 1, 4 * hg + 2 * u:4 * hg + 2 * u + 2,
                    :].rearrange("p u q -> p (u q)"),
                start=True, stop=True)
            nc.vector.tensor_copy(
                bcs[:, 4 * hg + 2 * u:4 * hg + 2 * u + 2, :],
                bc[:, 0:392].rearrange("p (u q) -> p u q", u=2))

    def norm_mults(bcs, ts2, todd, hg, o_of):
        for hh in range(4):
            h = 4 * hg + hh
            ot, osl = o_of[hh]
            dst = (ts2[0:VD, h // 2, :] if h % 2 == 0
                   else todd[:, h // 2, :])
            nc.vector.tensor_tensor(out=dst, in0=ot[0:VD, osl, 0:N2],
                                    in1=bcs[:, h, :], op=ALU.mult)

    def vl_units(vc, vl):
        """v_local dwconv emitted as single-matmul units so the in-order PE
        queue can interleave them finely with the attention chunks (PSUM
        accumulation groups tolerate interleaved matmuls to other banks)."""
        state = {}

        def mk_mm(t, ti):
            def emit():
                if ti == 0:
                    vlps = vp.tile([128, 256], F32, tag="vlp")
                    state[t] = vlps
                ps = state[t]
                a, bb = divmod(ti, 3)
                pr, pc = tap_geom(a), tap_geom(bb)
                off = (15 if a else 0) + (1 if bb else 0)
                nc.tensor.matmul(
                    ps[:, 0:209], vd[:, 9 * t + ti, :],
                    vc[:, t, 225 * (2 * pr + pc) + off:
                       225 * (2 * pr + pc) + off + 209],
                    start=(ti == 0), stop=(ti == 8))
            return emit

        def mk_retire(t):
            def emit():
                ps = state.pop(t)
                eng = nc.vector if t % 2 == 0 else nc.scalar
                if t % 2 == 0:
                    nc.vector.tensor_scalar_add(
                        vl[:, t, :].rearrange("p (r c) -> p r c", r=14),
                        ps[:, 0:210].rearrange(
                            "p (r c) -> p r c", c=15)[:, :, 0:14],
                        bias[:, 2 + t:3 + t])
                else:
                    nc.scalar.activation(
                        vl[:, t, :].rearrange("p (r c) -> p r c", r=14),
                        ps[:, 0:210].rearrange(
                            "p (r c) -> p r c", c=15)[:, :, 0:14],
                        AF.Identity, bias=bias[:, 2 + t:3 + t])
            return emit

        units = []
        for t in range(4):
            units.extend(mk_mm(t, ti) for ti in range(9))
            units.append(mk_retire(t))
        return units

    head_load(0)
    head_compute(0)
    for b in range(B_LOC):
        rec = sb.tile([VD + 1, NH, N2], BF16, tag="rec")
        bcs = sb.tile([VD, NH, N2], F32, tag="bcs")
        ts2 = sb.tile([128, 4, N2], BF16, tag="ts2")
        todd = sb.tile([VD, 4, N2], BF16, tag="todd")
        o_hg = []
        for hg in range(2):
            o_pa = op.tile([VD + 1, 2, 256], F32, tag="o")
            o_pb = op.tile([VD + 1, 2, 256], F32, tag="o")
            o_hg.append([(o_pa, 0), (o_pa, 1), (o_pb, 0), (o_pb, 1)])
        vc = vc_ab[b % 2]
        vl = sb.tile([128, 4, N2], BF16, tag="vl")
        # Software-pipelined emission: every pv_chunk trails its s_chunk by
        # one step, with independent PE work (vc transposes / vl dwconv /
        # norm bcasts / next-batch head) in between, so the in-order PE
        # queue never parks on the exp -> PV semaphore chain. The hg0 norm
        # (DVE-heavy) overlaps hg1's early chunks; the hg1 norm overlaps the
        # next batch's head compute.
        units = vl_units(vc, vl)
        ui = [0]

        def fill(k):
            while k > 0 and ui[0] < len(units):
                units[ui[0]]()
                ui[0] += 1
                k -= 1

        es0 = s_chunk(b, 0, 0)
        fill(1)
        for c in range(1, NCH):
            es1 = s_chunk(b, 0, c)
            fill(1)
            pv_chunk(b, 0, c - 1, es0, o_hg[0])
            fill(2)
            es0 = es1
        es10 = s_chunk(b, 1, 0)
        fill(1)
        pv_chunk(b, 0, NCH - 1, es0, o_hg[0])
        fill(2)
        norm_recips(rec, 0, o_hg[0])
        if b + 1 < B_LOC:
            head_load(b + 1)
        es11 = s_chunk(b, 1, 1)
        fill(1)
        norm_bcast(rec, bcs, 0)
        norm_mults(bcs, ts2, todd, 0, o_hg[0])
        pv_chunk(b, 1, 0, es10, o_hg[1])
        fill(2)
        es0 = es11
        for c in range(2, NCH):
            es1 = s_chunk(b, 1, c)
            fill(1)
            pv_chunk(b, 1, c - 1, es0, o_hg[1])
            fill(2)
            es0 = es1
        pv_chunk(b, 1, NCH - 1, es0, o_hg[1])
        fill(len(units))
        norm_recips(rec, 1, o_hg[1])
        if b + 1 < B_LOC:
            head_compute(b + 1)  # big independent PE block hides the norm
        norm_bcast(rec, bcs, 1)
        norm_mults(bcs, ts2, todd, 1, o_hg[1])

        nc.sync.dma_start(ts2[VD:128, :, :], todd[:])
        r_sb = sb.tile([128, 4, N2], BF16, tag="r")
        nc.vector.tensor_tensor(out=r_sb[:], in0=ts2[:], in1=vl[:],
                                op=ALU.add)
        nc.vector.tensor_scalar_max(r_sb[:], r_sb[:], 0.0)

        # output projection
        o_sb = sb.tile([128, 3, N2], BF16, tag="os")
        for m in range(3):
            ps = pp.tile([128, N2], F32, tag="proj")
            for j in range(4):
                nc.tensor.matmul(ps[:], pwT[:, j, 128 * m:128 * m + 128],
                                 r_sb[:, j, :], start=(j == 0), stop=(j == 3))
            nc.vector.tensor_scalar_add(o_sb[:, m, :], ps[:],
                                        bias[:, 6 + m:7 + m])
        nc.sync.dma_start(out_d[b], o_sb[:])
    ctx.close()


# ----------------------------------------------------------------------------
# persistent runner (AOT fast-dispatch PJRT executable + device consts)
# ----------------------------------------------------------------------------
class _State:
    pass


_STATE = None
_LAST_EXEC_NS = None


def last_exec_ns():
    return _LAST_EXEC_NS


def _ensure_state():
    global _STATE
    if _STATE is not None:
        return _STATE
    st = _State()
    nc = build_program()
    install_neuronx_cc_hook()
    partition_name = (nc.partition_id_tensor.name
                      if nc.partition_id_tensor else None)
    in_names, out_names, out_avals = [], [], []
    for alloc in nc.m.functions[0].allocations:
        if not isinstance(alloc, mybir.MemoryLocationSet):
            continue
        name = alloc.memorylocations[0].name
        if alloc.kind == "ExternalInput":
            if name != partition_name:
                in_names.append(name)
        elif alloc.kind == "ExternalOutput":
            out_names.append(name)
            out_avals.append(jax.core.ShapedArray(
                tuple(alloc.tensor_shape), mybir.dt.np(alloc.dtype)))
    assert in_names[0] == "x" and out_names == ["out"]
    all_in_names = list(in_names) + list(out_names)
    if partition_name is not None:
        all_in_names.append(partition_name)

    def _body(*args):
        operands = list(args)
        if partition_name is not None:
            operands.append(partition_id_tensor())
        outs = _bass_exec_p.bind(
            *operands, out_avals=tuple(out_avals),
            in_names=tuple(all_in_names), out_names=tuple(out_names),
            lowering_input_output_aliases=(),
            sim_require_finite=True, sim_require_nnan=True, nc=nc)
        return tuple(outs)

    devices = jax.devices()[:N_CORES]
    mesh = Mesh(np.asarray(devices), ("core",))
    sh = NamedSharding(mesh, PartitionSpec("core"))
    n_in = len(in_names)
    donate = (n_in,)  # the recycled output buffer
    in_specs = (PartitionSpec("core"),) * (n_in + 1)
    out_specs = (PartitionSpec("core"),)

    # global (concat-over-cores) shapes for every operand
    per_core_shapes = {}
    for alloc in nc.m.functions[0].allocations:
        if not isinstance(alloc, mybir.MemoryLocationSet):
            continue
        name = alloc.memorylocations[0].name
        if name in in_names or name in out_names:
            per_core_shapes[name] = (tuple(alloc.tensor_shape),
                                     np.dtype(mybir.dt.np(alloc.dtype)))

    def gshape(name):
        s, d = per_core_shapes[name]
        return jax.ShapeDtypeStruct((N_CORES * s[0], *s[1:]), d, sharding=sh)

    arg_specs = [gshape(nm) for nm in in_names] + [gshape("out")]

    def _compile():
        jitted = jax.jit(
            shard_map(_body, mesh=mesh, in_specs=in_specs,
                      out_specs=out_specs, check_rep=False),
            donate_argnums=donate, keep_unused=True)
        return jitted.lower(*arg_specs).compile()

    st.fn = fast_dispatch_compile(_compile)
    st.sh = sh
    st.in_names = in_names
    st.out_shape = per_core_shapes["out"]
    st.consts_key = None
    st.consts_dev = None
    st.prev_out = None
    _STATE = st
    return st


def _ensure_consts(st, inputs):
    key = [np.asarray(inputs[k]) for k in _WEIGHT_NAMES]
    if st.consts_key is not None and all(
            np.array_equal(a, b) for a, b in zip(st.consts_key, key)):
        return
    consts = _prep_consts(inputs)
    dev = []
    for nm in st.in_names[1:]:
        a = consts[nm]
        g = np.broadcast_to(a, (N_CORES, *a.shape)).reshape(
            N_CORES * a.shape[0], *a.shape[1:])
        dev.append(jax.device_put(np.ascontiguousarray(g), st.sh))
    jax.block_until_ready(dev)
    st.consts_dev = dev
    st.consts_key = key


def _pack_x(x4):
    """[64,C,28,28] f32 -> bf16 [64,128,3,784]: polyphase plane order along
    the last axis, channel split partition-major (p, j) so each partition's
    DMA run is one contiguous 4.7KB stretch."""
    xb = x4.astype(bf)
    x6 = xb.reshape(B, 3, 128, 14, 2, 14, 2)
    xr = x6.transpose(0, 2, 1, 4, 6, 3, 5)   # [B,p,j,pr,pc,r,c]
    return np.ascontiguousarray(xr).reshape(B, 128, 3, N)


def _fresh_outbuf(st):
    s, d = st.out_shape
    return jax.device_put(np.zeros((N_CORES * s[0], *s[1:]), d), st.sh)


def kernel(**inputs):
    global _LAST_EXEC_NS
    st = _ensure_state()
    _ensure_consts(st, inputs)
    xg = _pack_x(np.asarray(inputs["x"], np.float32))
    x_dev = jax.device_put(xg, st.sh)
    ob = st.prev_out if st.prev_out is not None else _fresh_outbuf(st)
    jax.block_until_ready([x_dev, ob])
    t0 = time.perf_counter()
    out, = st.fn(x_dev, *st.consts_dev, ob)
    out.block_until_ready()
    t1 = time.perf_counter()
    _LAST_EXEC_NS = int((t1 - t0) * 1e9)
    host = np.asarray(out)                    # [64, 128, 3, N2] bf16
    st.prev_out = out
    st.last_x = x_dev
    return np.ascontiguousarray(
        host.astype(np.float32).reshape(B, 128, 3, N2).transpose(0, 2, 1, 3)
    ).reshape(B, OUT, H2, W2)


def exec_pipeline(n):
    """Run the compiled NEFF n times back-to-back on device (inputs staged by
    the last kernel() call), one sync at the end. Output buffers are donation-
    chained: execute i reuses the buffer produced by execute i-2, so the chain
    runs entirely on device. Returns wall seconds for the n executes + sync."""
    st = _STATE
    assert st is not None and st.prev_out is not None, "call kernel() first"
    if getattr(st, "spare", None) is None:
        st.spare = _fresh_outbuf(st)
    bufs = [st.prev_out, st.spare]
    jax.block_until_ready([st.last_x, *bufs])
    t0 = time.perf_counter()
    for i in range(n):
        o, = st.fn(st.last_x, *st.consts_dev, bufs[i])
        bufs.append(o)
    bufs[-1].block_until_ready()
    t1 = time.perf_counter()
    # donated: bufs[0..n-1]; still live: bufs[n], bufs[n+1] (the last two)
    st.prev_out, st.spare = bufs[-1], bufs[-2]
    return t1 - t0

